# revision 14
# baseline (speedup 1.0000x reference)
import sys

import numpy as np

try:
    from concourse import bacc, bass, tile, masks
    from concourse.bass_utils import run_bass_kernel_spmd
except ImportError:
    sys.path.insert(0, "/opt/trn_rl_repo")
    from concourse import bacc, bass, tile, masks
    from concourse.bass_utils import run_bass_kernel_spmd

mybir = bass.mybir

N, D, F, H = 8192, 256, 256, 256
NC = 8
RPC = N // NC           # rows per core
TILES = RPC // 128      # 128-row tiles per core
LN_EPS = 1e-5
DENOM_EPS = 1e-8
FP = mybir.dt.float32
FPR = mybir.dt.float32r
BF = mybir.dt.bfloat16
AF = mybir.ActivationFunctionType
ALU = mybir.AluOpType
AX = mybir.AxisListType
SA = H + 2  # augmented cols padded even


def _build_kernel():
    nc = bacc.Bacc(None, target_bir_lowering=False)

    x_in = nc.declare_dram_parameter("x", [RPC, D], FP, isOutput=False)
    wqr_in = nc.declare_dram_parameter("wqr", [D, F], FPR, isOutput=False)
    wkr_in = nc.declare_dram_parameter("wkr", [D, F], FPR, isOutput=False)
    wv_in = nc.declare_dram_parameter("wv", [D, H], FPR, isOutput=False)
    w1_in = nc.declare_dram_parameter("w1", [D, H], BF, isOutput=False)
    w2_in = nc.declare_dram_parameter("w2", [H, D], BF, isOutput=False)
    out_ext = nc.declare_dram_parameter("out", [RPC, D], FP, isOutput=True)

    with tile.TileContext(nc) as tc:
        with (
            tc.tile_pool(name="const", bufs=1) as const_pool,
            tc.tile_pool(name="wpool", bufs=1) as wpool,
            tc.tile_pool(name="store", bufs=1) as store_pool,
            tc.tile_pool(name="dram", bufs=1, space="DRAM") as dram_pool,
        ):
            ident = const_pool.tile([128, 128], FP)
            masks.make_identity(nc, ident[:])
            eps_t = const_pool.tile([128, 1], FP)
            nc.vector.memset(eps_t[:], LN_EPS)
            ones_t = const_pool.tile([128, 2], BF)
            nc.vector.memset(ones_t[:], 1.0)
            ident_bf = const_pool.tile([128, 128], BF)
            nc.scalar.copy(ident_bf[:], ident[:])

            def load_w(dram_t, name, cols=256):
                t = wpool.tile([128, 2, cols], dram_t.dtype, name=name)
                for c in (0, 1):
                    nc.sync.dma_start(
                        out=t[:, c, :], in_=dram_t[c * 128 : (c + 1) * 128, :]
                    )
                return t

            # k-side weights first: they gate phase A1 start.
            wkr = load_w(wkr_in, "wkr_sb")
            wv = load_w(wv_in, "wv_sb")

            x_store = store_pool.tile([128, TILES, D], FP)
            for t in range(TILES):
                nc.sync.dma_start(
                    out=x_store[:, t, :], in_=x_in[t * 128 : (t + 1) * 128, :]
                )

            wqr = load_w(wqr_in, "wqr_sb")
            w1 = load_w(w1_in, "w1_sb")
            w2 = load_w(w2_in, "w2_sb")

            xT_store = store_pool.tile([128, TILES, 2, 128], FPR)
            eqT_store = store_pool.tile([128, TILES, 2, 128], BF)
            s_sb = store_pool.tile([128, 2, SA], BF)
            # Wo is folded into Wv host-side (wv = Wv@Wo), so the
            # AllReduced S is directly T_aug = [S_v@Wo | colsum].
            s_red = store_pool.tile([128, 2, SA], BF)

            # ---------------- Phase A1: k-side -> local S ---------------------
            with (
                tc.tile_pool(name="a_sb", bufs=3) as a_sb,
                tc.tile_pool(name="a_ps", bufs=2, space="PSUM") as a_ps,
                tc.tile_pool(name="s_ps", bufs=1, space="PSUM") as s_ps,
            ):
                s_psum = [
                    s_ps.tile([128, SA], FP, name=f"s_psum{c}") for c in (0, 1)
                ]

                ek_t = [None] * TILES
                va_t = [None] * TILES

                def a1_front(t):
                    xt_ps = a_ps.tile([128, 512], FP)
                    for c in (0, 1):
                        nc.tensor.transpose(
                            xt_ps[:, c * 128 : (c + 1) * 128],
                            x_store[:, t, c * 128 : (c + 1) * 128],
                            ident[:],
                        )
                        nc.vector.tensor_scalar_add(
                            xT_store[:, t, c, :],
                            xt_ps[:, c * 128 : (c + 1) * 128],
                            0.0,
                        )

                    # k cols 0:256, v cols 256:512 — ONE accumulation group
                    # (start zeroes the whole 2KB PSUM bank).
                    kv_ps = a_ps.tile([128, 512], FP)
                    for c in (0, 1):
                        nc.tensor.matmul(
                            kv_ps[:, 0:256], xT_store[:, t, c, :], wkr[:, c, :],
                            start=(c == 0), stop=False,
                        )
                        nc.tensor.matmul(
                            kv_ps[:, 256:512], xT_store[:, t, c, :], wv[:, c, :],
                            start=False, stop=(c == 1),
                        )

                    nmk = a_sb.tile([128, 1], FP)
                    nc.vector.tensor_reduce(
                        out=nmk[:], in_=kv_ps[:, 0:256], axis=AX.X, op=ALU.max,
                        negate=True,
                    )
                    ek = a_sb.tile([128, F], BF, name="ek_keep")
                    nc.scalar.activation(ek[:], kv_ps[:, 0:256], AF.Exp, bias=nmk[:])

                    v_aug = a_sb.tile([128, SA], BF, name="va_keep")
                    nc.scalar.copy(v_aug[:, 0:H], kv_ps[:, 256:512])
                    nc.scalar.copy(v_aug[:, H:SA], ones_t[:])
                    ek_t[t] = ek
                    va_t[t] = v_aug

                def a1_smm(t):
                    for c in (0, 1):
                        nc.tensor.matmul(
                            s_psum[c][:], ek_t[t][:, c * 128 : (c + 1) * 128],
                            va_t[t][:], start=(t == 0), stop=(t == TILES - 1),
                        )

                # one-tile skew: S matmuls never stall the tensor queue on exp
                for t in range(TILES):
                    a1_front(t)
                    if t >= 1:
                        a1_smm(t - 1)
                a1_smm(TILES - 1)

                for c in (0, 1):
                    nc.scalar.copy(s_sb[:, c, :], s_psum[c][:])

            # ---------------- AllReduce of S_aug across 8 cores --------------
            cc_in = dram_pool.tile([128, 2, SA], BF)
            cc_out = dram_pool.tile([128, 2, SA], BF, addr_space="Shared")
            nc.sync.dma_start(out=cc_in[:], in_=s_sb[:])
            nc.gpsimd.collective_compute(
                "AllReduce",
                ALU.add,
                replica_groups=[list(range(NC))],
                ins=[cc_in[:].opt()],
                outs=[cc_out[:].opt()],
            )
            # gpsimd queue: keeps this collective-blocked DMA off the sync
            # queue so nothing later queues behind the AllReduce.
            for c in (0, 1):
                nc.gpsimd.dma_start(out=s_red[:, c, :], in_=cc_out[:, c, :])

            # ---------------- Phase A2: q-side (runs under the AllReduce) ----
            with (
                tc.tile_pool(name="q_sb", bufs=3) as q_sb,
                tc.tile_pool(name="q_ps", bufs=2, space="PSUM") as q_ps,
            ):
                for t in range(TILES):
                    qp_ps = q_ps.tile([128, 256], FP)
                    for c in (0, 1):
                        nc.tensor.matmul(
                            qp_ps[:, 0:256], xT_store[:, t, c, :], wqr[:, c, :],
                            start=(c == 0), stop=(c == 1),
                        )
                    nmq = q_sb.tile([128, 1], FP)
                    nc.vector.tensor_reduce(
                        out=nmq[:], in_=qp_ps[:, 0:256], axis=AX.X, op=ALU.max,
                        negate=True,
                    )
                    eq = q_sb.tile([128, F], BF)
                    nc.scalar.activation(eq[:], qp_ps[:, 0:256], AF.Exp, bias=nmq[:])
                    # bf16 transpose via DMA crossbar (sync queue is clear now
                    # that the collective-blocked s_red DMA is on gpsimd)
                    nc.sync.dma_start_transpose(
                        out=eqT_store[:, t, :, :], in_=eq[:]
                    )

            # ---------------- Phase B: numer, LN1, FFN, LN2 ------------------
            # num accum gives rowsum via ones-cols; STT accum_out gives LN
            # means for free; variance via scalar Square+accum; normalize
            # split across vector (LN1) and scalar Identity (LN2).
            NPAIR = TILES // 2
            with (
                tc.tile_pool(name="b_sb", bufs=6) as b_sb,
                tc.tile_pool(name="p_num", bufs=2, space="PSUM") as p_num,
                tc.tile_pool(name="p_hT", bufs=2, space="PSUM") as p_hT,
                tc.tile_pool(name="p_ff1", bufs=2, space="PSUM") as p_ff1,
                tc.tile_pool(name="p_ff2", bufs=2, space="PSUM") as p_ff2,
            ):
                h_t = [None] * TILES
                hT_p = [None] * NPAIR
                f1T_p = [None] * NPAIR

                def stage1(t):
                    num_ps = p_num.tile([128, SA], FP, name="num_ps")
                    for c in (0, 1):
                        nc.tensor.matmul(
                            num_ps[:], eqT_store[:, t, c, :], s_red[:, c, :],
                            start=(c == 0), stop=(c == 1),
                        )
                    d_sb = b_sb.tile([128, 1], FP)
                    r = b_sb.tile([128, 1], FP)
                    nc.vector.tensor_scalar_add(
                        d_sb[:], num_ps[:, H : H + 1], DENOM_EPS
                    )
                    nc.vector.reciprocal(r[:], d_sb[:])
                    hin = b_sb.tile([128, D], FP)
                    hsum = b_sb.tile([128, 1], FP)
                    nc.vector.scalar_tensor_tensor(
                        out=hin[:],
                        in0=num_ps[:, 0:D],
                        scalar=r[:],
                        in1=x_store[:, t, :],
                        op0=ALU.mult,
                        op1=ALU.add,
                        accum_out=hsum[:],
                    )
                    m1 = b_sb.tile([128, 1], FP)
                    nc.vector.tensor_scalar_mul(m1[:], hsum[:], 1.0 / D)
                    # var: scalar Square pass, accum -> sum((m1-hin)^2)
                    junk = b_sb.tile([128, D], BF, name="junk1")
                    vs1 = b_sb.tile([128, 1], FP)
                    nc.scalar.activation(
                        junk[:], hin[:], AF.Square, bias=m1[:], scale=-1.0,
                        accum_out=vs1[:],
                    )
                    std1 = b_sb.tile([128, 1], FP)
                    nc.scalar.activation(
                        std1[:], vs1[:], AF.Sqrt, bias=eps_t[:], scale=1.0 / D
                    )
                    rstd1 = b_sb.tile([128, 1], FP)
                    nc.vector.reciprocal(rstd1[:], std1[:])
                    h = b_sb.tile([128, D], BF, name="h_keep")
                    # normalize on gpsimd (SBUF-only op, engine otherwise idle)
                    nc.gpsimd.tensor_scalar(
                        h[:], hin[:], m1[:], rstd1[:], ALU.subtract, ALU.mult
                    )
                    h_t[t] = h

                def s2_transpose(p):
                    hT_ps = p_hT.tile([128, 512], BF, name="hT_ps")
                    hT2 = b_sb.tile([128, 2, 256], BF, name="hT2")
                    for j in (0, 1):
                        for c in (0, 1):
                            k = 2 * j + c
                            nc.tensor.transpose(
                                hT_ps[:, k * 128 : (k + 1) * 128],
                                h_t[2 * p + j][:, c * 128 : (c + 1) * 128],
                                ident_bf[:],
                            )
                            nc.scalar.copy(
                                hT2[:, c, j * 128 : (j + 1) * 128],
                                hT_ps[:, k * 128 : (k + 1) * 128],
                            )
                    hT_p[p] = hT2

                def s2_ffn1(p):
                    pre1T = p_ff1.tile([128, 512], FP, name="pre1T")
                    hT2 = hT_p[p]
                    # one merged accumulation group (start zeroes whole bank)
                    first = True
                    for fc in (0, 1):
                        for m in (0, 1):
                            nc.tensor.matmul(
                                pre1T[:, m * 256 : (m + 1) * 256],
                                w1[:, fc, m * 128 : (m + 1) * 128],
                                hT2[:, fc, :],
                                start=first, stop=(fc == 1 and m == 1),
                            )
                            first = False
                    f1T = b_sb.tile([128, 2, 256], BF, name="f1T2")
                    for m in (0, 1):
                        nc.scalar.activation(
                            f1T[:, m, :], pre1T[:, m * 256 : (m + 1) * 256], AF.Relu
                        )
                    f1T_p[p] = f1T

                def stage3(t):
                    p, j = t // 2, t % 2
                    f1T = f1T_p[p]
                    ff2_ps = p_ff2.tile([128, D], FP, name="ff2_ps")
                    for m in (0, 1):
                        nc.tensor.matmul(
                            ff2_ps[:], f1T[:, m, j * 128 : (j + 1) * 128],
                            w2[:, m, :], start=(m == 0), stop=(m == 1),
                        )
                    y2 = b_sb.tile([128, D], FP)
                    nc.vector.scalar_tensor_tensor(
                        out=y2[:], in0=ff2_ps[:], scalar=0.0, in1=h_t[t][:],
                        op0=ALU.bypass, op1=ALU.add,
                    )
                    # LN2 stats via bn_stats (vector), normalize on gpsimd
                    stats = b_sb.tile([128, 6], FP)
                    aggr = b_sb.tile([128, 2], FP)
                    nc.vector.bn_stats(stats[:], y2[:])
                    nc.vector.bn_aggr(aggr[:], stats[:])
                    std2 = b_sb.tile([128, 1], FP)
                    nc.scalar.activation(
                        std2[:], aggr[:, 1:2], AF.Sqrt, bias=eps_t[:]
                    )
                    rstd2 = b_sb.tile([128, 1], FP)
                    nc.vector.reciprocal(rstd2[:], std2[:])
                    outt = b_sb.tile([128, D], FP)
                    nc.gpsimd.tensor_scalar(
                        outt[:], y2[:], aggr[:, 0:1], rstd2[:],
                        ALU.subtract, ALU.mult,
                    )
                    nc.sync.dma_start(
                        out=out_ext[t * 128 : (t + 1) * 128, :], in_=outt[:]
                    )

                for i in range(NPAIR + 2):
                    if i < NPAIR:
                        stage1(2 * i)
                        stage1(2 * i + 1)
                    if 1 <= i <= NPAIR:
                        s2_transpose(i - 1)
                    if i >= 2:
                        stage3(2 * (i - 2))
                        stage3(2 * (i - 2) + 1)
                    if 1 <= i <= NPAIR:
                        s2_ffn1(i - 1)

    nc.finalize()
    return nc


_NC_CACHE = {}


def _get_nc():
    if "nc" not in _NC_CACHE:
        _NC_CACHE["nc"] = _build_kernel()
    return _NC_CACHE["nc"]


def _run(inputs, trace=False, **kw):
    import ml_dtypes

    x = np.ascontiguousarray(inputs["x"], dtype=np.float32)
    R = inputs["R"].astype(np.float64)
    wqr = (inputs["Wq"].astype(np.float64) @ R).astype(np.float32)
    wkr = (inputs["Wk"].astype(np.float64) @ R).astype(np.float32)
    wvo = (
        inputs["Wv"].astype(np.float64) @ inputs["Wo"].astype(np.float64)
    ).astype(np.float32)
    shared = {
        "wqr": np.ascontiguousarray(wqr),
        "wkr": np.ascontiguousarray(wkr),
        "wv": np.ascontiguousarray(wvo),
        "w1": np.ascontiguousarray(inputs["W1"].astype(ml_dtypes.bfloat16)),
        "w2": np.ascontiguousarray(inputs["W2"].astype(ml_dtypes.bfloat16)),
    }
    in_maps = [
        {"x": np.ascontiguousarray(x[c * RPC : (c + 1) * RPC]), **shared}
        for c in range(NC)
    ]
    nc = _get_nc()
    res = run_bass_kernel_spmd(nc, in_maps, list(range(NC)), trace=trace, **kw)
    out = np.concatenate([res.results[c]["out"] for c in range(NC)], axis=0)
    return out.astype(np.float32), res


def kernel(**inputs) -> np.ndarray:
    out, _ = _run(inputs)
    return out


# revision 20
# speedup vs baseline: 1.9898x; 1.9898x over previous
import sys

import numpy as np

try:
    from concourse import bacc, bass, tile, masks
    from concourse.bass_utils import run_bass_kernel_spmd
except ImportError:
    sys.path.insert(0, "/opt/trn_rl_repo")
    from concourse import bacc, bass, tile, masks
    from concourse.bass_utils import run_bass_kernel_spmd

mybir = bass.mybir

N, D, F, H = 8192, 256, 256, 256
NC = 8
RPC = N // NC           # rows per core
TILES = RPC // 128      # 128-row tiles per core
LN_EPS = 1e-5
DENOM_EPS = 1e-8
FP = mybir.dt.float32
FPR = mybir.dt.float32r
BF = mybir.dt.bfloat16
AF = mybir.ActivationFunctionType
ALU = mybir.AluOpType
AX = mybir.AxisListType
SA = H + 2  # augmented cols padded even


def _build_kernel():
    nc = bacc.Bacc(None, target_bir_lowering=False)

    x_in = nc.declare_dram_parameter("x", [RPC, D], FP, isOutput=False)
    wqr_in = nc.declare_dram_parameter("wqr", [D, F], FPR, isOutput=False)
    wkr_in = nc.declare_dram_parameter("wkr", [D, F], FPR, isOutput=False)
    wv_in = nc.declare_dram_parameter("wv", [D, H], FPR, isOutput=False)
    w1_in = nc.declare_dram_parameter("w1", [D, H], BF, isOutput=False)
    w2_in = nc.declare_dram_parameter("w2", [H, D], BF, isOutput=False)
    out_ext = nc.declare_dram_parameter("out", [RPC, D], FP, isOutput=True)

    with tile.TileContext(nc) as tc:
        with (
            tc.tile_pool(name="const", bufs=1) as const_pool,
            tc.tile_pool(name="wpool", bufs=1) as wpool,
            tc.tile_pool(name="store", bufs=1) as store_pool,
            tc.tile_pool(name="dram", bufs=1, space="DRAM") as dram_pool,
        ):
            ident = const_pool.tile([128, 128], FP)
            masks.make_identity(nc, ident[:])
            eps_t = const_pool.tile([128, 1], FP)
            nc.vector.memset(eps_t[:], LN_EPS)
            ones_t = const_pool.tile([128, 2], BF)
            nc.vector.memset(ones_t[:], 1.0)
            ident_bf = const_pool.tile([128, 128], BF)
            nc.scalar.copy(ident_bf[:], ident[:])

            def load_w(dram_t, name, cols=256):
                t = wpool.tile([128, 2, cols], dram_t.dtype, name=name)
                for c in (0, 1):
                    nc.sync.dma_start(
                        out=t[:, c, :], in_=dram_t[c * 128 : (c + 1) * 128, :]
                    )
                return t

            # k-side weights first: they gate phase A1 start.
            wkr = load_w(wkr_in, "wkr_sb")
            wv = load_w(wv_in, "wv_sb")

            x_store = store_pool.tile([128, TILES, D], FP)
            for t in range(TILES):
                nc.sync.dma_start(
                    out=x_store[:, t, :], in_=x_in[t * 128 : (t + 1) * 128, :]
                )

            wqr = load_w(wqr_in, "wqr_sb")
            w1 = load_w(w1_in, "w1_sb")
            w2 = load_w(w2_in, "w2_sb")

            xT_store = store_pool.tile([128, TILES, 2, 128], FPR)
            eqT_store = store_pool.tile([128, TILES, 2, 128], BF)
            s_sb = store_pool.tile([128, 2, SA], BF)
            # Wo is folded into Wv host-side (wv = Wv@Wo), so the
            # AllReduced S is directly T_aug = [S_v@Wo | colsum].
            s_red = store_pool.tile([128, 2, SA], BF)

            # ---------------- Phase A1: k-side -> local S ---------------------
            with (
                tc.tile_pool(name="a_sb", bufs=3) as a_sb,
                tc.tile_pool(name="a_ps", bufs=2, space="PSUM") as a_ps,
                tc.tile_pool(name="s_ps", bufs=1, space="PSUM") as s_ps,
            ):
                s_psum = [
                    s_ps.tile([128, SA], FP, name=f"s_psum{c}") for c in (0, 1)
                ]

                ek_t = [None] * TILES
                va_t = [None] * TILES

                def a1_front(t):
                    xt_ps = a_ps.tile([128, 512], FP)
                    for c in (0, 1):
                        nc.tensor.transpose(
                            xt_ps[:, c * 128 : (c + 1) * 128],
                            x_store[:, t, c * 128 : (c + 1) * 128],
                            ident[:],
                        )
                        nc.vector.tensor_scalar_add(
                            xT_store[:, t, c, :],
                            xt_ps[:, c * 128 : (c + 1) * 128],
                            0.0,
                        )

                    # k cols 0:256, v cols 256:512 — ONE accumulation group
                    # (start zeroes the whole 2KB PSUM bank).
                    kv_ps = a_ps.tile([128, 512], FP)
                    for c in (0, 1):
                        nc.tensor.matmul(
                            kv_ps[:, 0:256], xT_store[:, t, c, :], wkr[:, c, :],
                            start=(c == 0), stop=False,
                        )
                        nc.tensor.matmul(
                            kv_ps[:, 256:512], xT_store[:, t, c, :], wv[:, c, :],
                            start=False, stop=(c == 1),
                        )

                    nmk = a_sb.tile([128, 1], FP)
                    nc.vector.tensor_reduce(
                        out=nmk[:], in_=kv_ps[:, 0:256], axis=AX.X, op=ALU.max,
                        negate=True,
                    )
                    ek = a_sb.tile([128, F], BF, name="ek_keep")
                    nc.scalar.activation(ek[:], kv_ps[:, 0:256], AF.Exp, bias=nmk[:])

                    v_aug = a_sb.tile([128, SA], BF, name="va_keep")
                    nc.scalar.copy(v_aug[:, 0:H], kv_ps[:, 256:512])
                    nc.scalar.copy(v_aug[:, H:SA], ones_t[:])
                    ek_t[t] = ek
                    va_t[t] = v_aug

                def a1_smm(t):
                    for c in (0, 1):
                        nc.tensor.matmul(
                            s_psum[c][:], ek_t[t][:, c * 128 : (c + 1) * 128],
                            va_t[t][:], start=(t == 0), stop=(t == TILES - 1),
                        )

                # one-tile skew: S matmuls never stall the tensor queue on exp
                for t in range(TILES):
                    a1_front(t)
                    if t >= 1:
                        a1_smm(t - 1)
                a1_smm(TILES - 1)

                for c in (0, 1):
                    nc.scalar.copy(s_sb[:, c, :], s_psum[c][:])

            # ---------------- AllReduce of S_aug across 8 cores --------------
            cc_in = dram_pool.tile([128, 2, SA], BF)
            cc_out = dram_pool.tile([128, 2, SA], BF, addr_space="Shared")
            nc.sync.dma_start(out=cc_in[:], in_=s_sb[:])
            nc.gpsimd.collective_compute(
                "AllReduce",
                ALU.add,
                replica_groups=[list(range(NC))],
                ins=[cc_in[:].opt()],
                outs=[cc_out[:].opt()],
            )
            # ---------------- Phase A2: q-side (runs under the AllReduce) ----
            with (
                tc.tile_pool(name="q_sb", bufs=3) as q_sb,
                tc.tile_pool(name="q_ps", bufs=2, space="PSUM") as q_ps,
            ):
                for t in range(TILES):
                    qp_ps = q_ps.tile([128, 256], FP)
                    for c in (0, 1):
                        nc.tensor.matmul(
                            qp_ps[:, 0:256], xT_store[:, t, c, :], wqr[:, c, :],
                            start=(c == 0), stop=(c == 1),
                        )
                    nmq = q_sb.tile([128, 1], FP)
                    nc.vector.tensor_reduce(
                        out=nmq[:], in_=qp_ps[:, 0:256], axis=AX.X, op=ALU.max,
                        negate=True,
                    )
                    eq = q_sb.tile([128, F], BF)
                    nc.scalar.activation(eq[:], qp_ps[:, 0:256], AF.Exp, bias=nmq[:])
                    # bf16 transpose via DMA crossbar (sync queue is clear now
                    # that the collective-blocked s_red DMA is on gpsimd)
                    nc.sync.dma_start_transpose(
                        out=eqT_store[:, t, :, :], in_=eq[:]
                    )

            # s_red DMA emitted AFTER A2 so A2's sync-queue transposes are
            # not stuck behind this collective-blocked transfer.
            for c in (0, 1):
                nc.sync.dma_start(out=s_red[:, c, :], in_=cc_out[:, c, :])

            # ---------------- Phase B: numer, LN1, FFN, LN2 ------------------
            # num accum gives rowsum via ones-cols; STT accum_out gives LN
            # means for free; variance via scalar Square+accum; normalize
            # split across vector (LN1) and scalar Identity (LN2).
            NPAIR = TILES // 2
            with (
                tc.tile_pool(name="b_sb", bufs=6) as b_sb,
                tc.tile_pool(name="p_num", bufs=2, space="PSUM") as p_num,
                tc.tile_pool(name="p_ff1", bufs=2, space="PSUM") as p_ff1,
                tc.tile_pool(name="p_ff2", bufs=2, space="PSUM") as p_ff2,
            ):
                h_t = [None] * TILES
                hT_p = [None] * NPAIR
                f1T_p = [None] * NPAIR

                def stage1(t):
                    num_ps = p_num.tile([128, SA], FP, name="num_ps")
                    for c in (0, 1):
                        nc.tensor.matmul(
                            num_ps[:], eqT_store[:, t, c, :], s_red[:, c, :],
                            start=(c == 0), stop=(c == 1),
                        )
                    d_sb = b_sb.tile([128, 1], FP)
                    r = b_sb.tile([128, 1], FP)
                    nc.vector.tensor_scalar_add(
                        d_sb[:], num_ps[:, H : H + 1], DENOM_EPS
                    )
                    nc.vector.reciprocal(r[:], d_sb[:])
                    hin = b_sb.tile([128, D], FP)
                    hsum = b_sb.tile([128, 1], FP)
                    nc.vector.scalar_tensor_tensor(
                        out=hin[:],
                        in0=num_ps[:, 0:D],
                        scalar=r[:],
                        in1=x_store[:, t, :],
                        op0=ALU.mult,
                        op1=ALU.add,
                        accum_out=hsum[:],
                    )
                    m1 = b_sb.tile([128, 1], FP)
                    nc.vector.tensor_scalar_mul(m1[:], hsum[:], 1.0 / D)
                    # var: scalar Square pass, accum -> sum((m1-hin)^2)
                    junk = b_sb.tile([128, D], BF, name="junk1")
                    vs1 = b_sb.tile([128, 1], FP)
                    nc.scalar.activation(
                        junk[:], hin[:], AF.Square, bias=m1[:], scale=-1.0,
                        accum_out=vs1[:],
                    )
                    std1 = b_sb.tile([128, 1], FP)
                    nc.scalar.activation(
                        std1[:], vs1[:], AF.Sqrt, bias=eps_t[:], scale=1.0 / D
                    )
                    rstd1 = b_sb.tile([128, 1], FP)
                    nc.vector.reciprocal(rstd1[:], std1[:])
                    h = b_sb.tile([128, D], BF, name="h_keep")
                    nc.vector.tensor_scalar(
                        h[:], hin[:], m1[:], rstd1[:], ALU.subtract, ALU.mult
                    )
                    h_t[t] = h

                def s2_transpose(p):
                    hT2 = b_sb.tile([128, 2, 256], BF, name="hT2")
                    for j in (0, 1):
                        eng = nc.sync if j == 0 else nc.scalar
                        eng.dma_start_transpose(
                            out=hT2[:, :, j * 128 : (j + 1) * 128],
                            in_=h_t[2 * p + j][:],
                        )
                    hT_p[p] = hT2

                def s2_ffn1(p):
                    pre1T = p_ff1.tile([128, 512], FP, name="pre1T")
                    hT2 = hT_p[p]
                    # one merged accumulation group (start zeroes whole bank)
                    first = True
                    for fc in (0, 1):
                        for m in (0, 1):
                            nc.tensor.matmul(
                                pre1T[:, m * 256 : (m + 1) * 256],
                                w1[:, fc, m * 128 : (m + 1) * 128],
                                hT2[:, fc, :],
                                start=first, stop=(fc == 1 and m == 1),
                            )
                            first = False
                    f1T = b_sb.tile([128, 2, 256], BF, name="f1T2")
                    for m in (0, 1):
                        nc.scalar.activation(
                            f1T[:, m, :], pre1T[:, m * 256 : (m + 1) * 256], AF.Relu
                        )
                    f1T_p[p] = f1T

                def stage3(t):
                    p, j = t // 2, t % 2
                    f1T = f1T_p[p]
                    ff2_ps = p_ff2.tile([128, D], FP, name="ff2_ps")
                    for m in (0, 1):
                        nc.tensor.matmul(
                            ff2_ps[:], f1T[:, m, j * 128 : (j + 1) * 128],
                            w2[:, m, :], start=(m == 0), stop=(m == 1),
                        )
                    y2 = b_sb.tile([128, D], FP)
                    ysum = b_sb.tile([128, 1], FP)
                    nc.vector.scalar_tensor_tensor(
                        out=y2[:], in0=ff2_ps[:], scalar=0.0, in1=h_t[t][:],
                        op0=ALU.bypass, op1=ALU.add,
                        accum_out=ysum[:],
                    )
                    m2 = b_sb.tile([128, 1], FP)
                    nc.vector.tensor_scalar_mul(m2[:], ysum[:], 1.0 / D)
                    junk2 = b_sb.tile([128, D], BF, name="junk2")
                    vs2 = b_sb.tile([128, 1], FP)
                    nc.scalar.activation(
                        junk2[:], y2[:], AF.Square, bias=m2[:], scale=-1.0,
                        accum_out=vs2[:],
                    )
                    std2 = b_sb.tile([128, 1], FP)
                    nc.scalar.activation(
                        std2[:], vs2[:], AF.Sqrt, bias=eps_t[:], scale=1.0 / D
                    )
                    rstd2 = b_sb.tile([128, 1], FP)
                    nc.vector.reciprocal(rstd2[:], std2[:])
                    outt = b_sb.tile([128, D], FP)
                    nc.vector.tensor_scalar(
                        outt[:], y2[:], m2[:], rstd2[:], ALU.subtract, ALU.mult
                    )
                    nc.sync.dma_start(
                        out=out_ext[t * 128 : (t + 1) * 128, :], in_=outt[:]
                    )

                for i in range(NPAIR + 2):
                    if i < NPAIR:
                        stage1(2 * i)
                        stage1(2 * i + 1)
                    if 1 <= i <= NPAIR:
                        s2_transpose(i - 1)
                    if i >= 2:
                        stage3(2 * (i - 2))
                        stage3(2 * (i - 2) + 1)
                    if 1 <= i <= NPAIR:
                        s2_ffn1(i - 1)

    nc.finalize()
    return nc


_NC_CACHE = {}


def _get_nc():
    if "nc" not in _NC_CACHE:
        _NC_CACHE["nc"] = _build_kernel()
    return _NC_CACHE["nc"]


def _run(inputs, trace=False, **kw):
    import ml_dtypes

    x = np.ascontiguousarray(inputs["x"], dtype=np.float32)
    R = inputs["R"].astype(np.float64)
    wqr = (inputs["Wq"].astype(np.float64) @ R).astype(np.float32)
    wkr = (inputs["Wk"].astype(np.float64) @ R).astype(np.float32)
    wvo = (
        inputs["Wv"].astype(np.float64) @ inputs["Wo"].astype(np.float64)
    ).astype(np.float32)
    shared = {
        "wqr": np.ascontiguousarray(wqr),
        "wkr": np.ascontiguousarray(wkr),
        "wv": np.ascontiguousarray(wvo),
        "w1": np.ascontiguousarray(inputs["W1"].astype(ml_dtypes.bfloat16)),
        "w2": np.ascontiguousarray(inputs["W2"].astype(ml_dtypes.bfloat16)),
    }
    in_maps = [
        {"x": np.ascontiguousarray(x[c * RPC : (c + 1) * RPC]), **shared}
        for c in range(NC)
    ]
    nc = _get_nc()
    res = run_bass_kernel_spmd(nc, in_maps, list(range(NC)), trace=trace, **kw)
    out = np.concatenate([res.results[c]["out"] for c in range(NC)], axis=0)
    return out.astype(np.float32), res


def kernel(**inputs) -> np.ndarray:
    out, _ = _run(inputs)
    return out
